# revision 23
# baseline (speedup 1.0000x reference)
"""Trainium2 Bass kernel for nn_GeneralizedAttention (Performer-style linear
attention with GELU random features).

Math (per (b,h)):
    qp  = gelu(q @ proj^T)            [n, m]
    kp  = gelu(k @ proj^T)            [n, m]
    ksum= kp.sum(n)                   [m]
    ctx = kp^T @ v                    [m, e]
    den = qp @ ksum                   [n]
    out = (qp @ ctx) / den[:, None]   [n, e]

Sharding: B*H = 64 (b,h) pairs split across 8 cores, 8 pairs each; proj_mat
replicated; no cross-core comms.

The ScalarE gelu stream (2 * n * m elements per pair at 1 elem/cycle/lane,
~18.3 us per pair in 1024-col instructions) is the roofline for this shape.
All engines execute their queues in order, so the emission order is software-
pipelined across (b,h) stages to keep ScalarE saturated:
  - the input transposes for stage s+1 are emitted mid-stage s (they only
    depend on the DMA'd inputs), so the projection matmuls of s+1 are ready
    the moment the last gelu of stage s retires;
  - the final context-accumulation chunk + ctx^T (gated on the last gelu of
    stage s) are deferred past the first qp blocks of stage s+1;
  - the 8 output groups of stage s-1 are interleaved between the kp blocks
    of stage s, filling PE idle windows under the gelu stream.

On-chip layouts per (b,h):
    q^T, k^T as [128, 16, 128] where partition = (t*64 + d), free = (j, p),
    n = j*256 + t*128 + p.  Both 64-row halves are used, so projection matmuls
    issue in (t=0, t=1) pairs on disjoint PE row groups and overlap.
    qp^T is kept [m, n]-major (feeds the final contraction over m),
    kp is kept [n, m]-major (feeds the context contraction over n).
    The ones column PREPENDED to v folds ksum/den into ctx/out as row 0 (the
    approx-reciprocal custom DVE op needs its input at partition 0).
    out^T = ctx_aug^T @ qp^T per 512-col tile (row 0 = den), normalized
    in the [e, n] layout (reciprocal_approx_fast + GpSimd partition
    broadcast + one DVE multiply) and stored as out^T tiles; the final
    [e, n] -> [n, e] layout permutation happens host-side in the unshard.
"""

import numpy as np

B, H, N, D, M = 4, 16, 4096, 64, 256
NCORES = 8
BH = B * H
BHPC = BH // NCORES  # 8 (b,h) pairs per core
P = 128
NJ = N // 256        # 16 pair-blocks of 256 n
NCH = N // P         # 32 chunks of 128 n
EAUG = D + 1         # 65: e plus the folded ksum/den row
EPAD = 66            # 4-byte aligned psum stride


def _emit_body(ctx, tc, out_d, q_d, k_d, v_d, proj_d, bhpc, repeat=1):
    import concourse.bass as bass
    import concourse.mybir as mybir
    from concourse.masks import make_identity

    nc = tc.nc
    f32 = mybir.dt.float32
    bf16 = mybir.dt.bfloat16
    MULT = mybir.AluOpType.mult
    GELU = mybir.ActivationFunctionType.Gelu

    const = ctx.enter_context(tc.tile_pool(name="const", bufs=1))
    inp = ctx.enter_context(tc.tile_pool(name="inp", bufs=6))
    vpool = ctx.enter_context(tc.tile_pool(name="vpool", bufs=3))
    tsb = ctx.enter_context(tc.tile_pool(name="tsb", bufs=6))
    feat = ctx.enter_context(tc.tile_pool(name="feat", bufs=2))
    small = ctx.enter_context(tc.tile_pool(name="small", bufs=3))
    outp = ctx.enter_context(tc.tile_pool(name="outp", bufs=3))
    ps_gen = ctx.enter_context(tc.tile_pool(name="ps_gen", bufs=2, space="PSUM"))
    ps_small = ctx.enter_context(tc.tile_pool(name="ps_small", bufs=1, space="PSUM"))
    ps_acc = ctx.enter_context(tc.tile_pool(name="ps_acc", bufs=1, space="PSUM"))
    ps_fin = ctx.enter_context(tc.tile_pool(name="ps_fin", bufs=2, space="PSUM"))

    ident_bf = const.tile([P, P], bf16, name="ident_bf")
    make_identity(nc, ident_bf)
    ident_f32 = const.tile([P, P], f32, name="ident_f32")
    make_identity(nc, ident_f32)

    # proj^T [d, m] duplicated on both partition halves (rows 0-63 and 64-127)
    proj_nat = const.tile([P, 2, D], f32, name="proj_nat")
    nc.sync.dma_start(proj_nat[:], proj_d.rearrange("(t p) d -> p t d", p=P))
    projT = const.tile([P, M], bf16, name="projT")
    for t in range(2):
        pspt = ps_small.tile([D, P], f32, tag="small", name=f"ps_projT{t}")
        nc.tensor.transpose(pspt[:], proj_nat[:, t, :], ident_f32)
        nc.vector.tensor_copy(projT[0:D, P * t : P * (t + 1)], pspt[:])
        nc.vector.tensor_copy(projT[D:P, P * t : P * (t + 1)], pspt[:])

    if repeat > 1:
        loop_cm = tc.For_i(
            0,
            repeat,
            1,
            hint_engines=(
                mybir.EngineType.PE,
                mybir.EngineType.DVE,
                mybir.EngineType.Activation,
            ),
        )
        loop_cm.__enter__()

    # ---------------- per-stage state ----------------
    qT_s, kT_s = [None] * bhpc, [None] * bhpc
    qpT_s, kp_s = [None] * bhpc, [None] * bhpc
    vaug_s, acc_s = [None] * bhpc, [None] * bhpc
    ctxT_s = [None] * bhpc
    ostage_s = [None] * bhpc

    def emit_loads(s):
        q_pairs = inp.tile([P, NJ, P], bf16, tag="qk", name=f"q_pairs{s}")
        nc.gpsimd.dma_start(
            q_pairs.rearrange("p j (t d) -> p j t d", t=2),
            q_d[s].rearrange("(j t p) d -> p j t d", t=2, p=P),
        )
        k_pairs = inp.tile([P, NJ, P], bf16, tag="qk", name=f"k_pairs{s}")
        nc.gpsimd.dma_start(
            k_pairs.rearrange("p j (t d) -> p j t d", t=2),
            k_d[s].rearrange("(j t p) d -> p j t d", t=2, p=P),
        )
        # ones column FIRST (index 0) so ksum/den land on partition 0 of the
        # ctx/out tiles: reciprocal_approx_fast requires a partition-0 input
        # (custom DVE ops mishandle nonzero input base partitions on HW)
        v_aug = vpool.tile([P, NCH, EAUG], bf16, tag="va", name=f"v_aug{s}")
        nc.gpsimd.memset(v_aug[:, :, 0:1], 1.0)
        nc.gpsimd.dma_start(
            v_aug[:, :, 1:EAUG], v_d[s].rearrange("(c p) d -> p c d", p=P)
        )
        vaug_s[s] = v_aug
        return q_pairs, k_pairs

    def emit_transpose_group(s, pairs, which, g):
        if which == "q" and g == 0:
            qT_s[s] = tsb.tile([P, NJ, P], bf16, tag="t", name=f"qT{s}")
        if which == "k" and g == 0:
            kT_s[s] = tsb.tile([P, NJ, P], bf16, tag="t", name=f"kT{s}")
        dst = qT_s[s] if which == "q" else kT_s[s]
        pst = ps_small.tile([P, 8, P], bf16, tag="small", name=f"ps_t{s}{which}{g}")
        for i in range(8):
            nc.tensor.transpose(pst[:, i, :], pairs[:, 8 * g + i, :], ident_bf)
        nc.vector.tensor_copy(dst[:, 8 * g : 8 * g + 8, :], pst[:])

    def emit_qp_block(s, mc, b4):
        if b4 == 0 and mc == 0:
            qpT_s[s] = feat.tile([P, 2, 2, NJ, P], bf16, tag="qpT", name=f"qpT{s}")
        qpT = qpT_s[s]
        psq = ps_gen.tile([P, 1024], f32, tag="gen", name=f"ps_qp{s}_{mc}{b4}")
        for t in range(2):
            nc.tensor.matmul(
                psq[:, 512 * t : 512 * (t + 1)],
                lhsT=projT[64 * t : 64 * t + 64, P * mc : P * (mc + 1)],
                rhs=qT_s[s][64 * t : 64 * t + 64, 4 * b4 : 4 * b4 + 4, :],
            )
        nc.scalar.activation(qpT[:, mc, :, 4 * b4 : 4 * b4 + 4, :], psq[:], GELU)

    def emit_kp_block(s, g):
        if g == 0:
            kp_s[s] = feat.tile([P, NCH, M], bf16, tag="kp", name=f"kp{s}")
            acc_s[s] = ps_acc.tile([EAUG, M], f32, tag="acc", name=f"ps_ctx{s}")
        kp = kp_s[s]
        kp_v = kp.rearrange("p (j t) m -> p t j m", t=2)
        psk = ps_gen.tile([P, 1024], f32, tag="gen", name=f"ps_kp{s}_{g}")
        for jl in range(2):
            j = 2 * g + jl
            for t in range(2):
                nc.tensor.matmul(
                    psk[:, 256 * (2 * t + jl) : 256 * (2 * t + jl + 1)],
                    lhsT=kT_s[s][64 * t : 64 * t + 64, j, :],
                    rhs=projT[64 * t : 64 * t + 64, :],
                )
        nc.scalar.activation(kp_v[:, :, 2 * g : 2 * g + 2, :], psk[:], GELU)
        # ctx MMs for this block's chunks; the last block's are deferred to
        # the next stage so they don't block the PE queue at the boundary
        if g < 7:
            emit_ctx_chunks(s, 4 * g, 4 * g + 4)

    def emit_ctx_chunks(s, c0, c1):
        for c in range(c0, c1):
            nc.tensor.matmul(
                acc_s[s][:],
                lhsT=vaug_s[s][:, c, :],
                rhs=kp_s[s][:, c, :],
                start=(c == 0),
                stop=(c == NCH - 1),
            )

    def emit_ctxT(s):
        emit_ctx_chunks(s, 28, 32)
        ctx_sb = small.tile([EAUG, M], bf16, tag="ctxsb", name=f"ctx_sb{s}")
        nc.vector.tensor_copy(ctx_sb[:], acc_s[s][:])
        psct = ps_small.tile([P, 2, EPAD], bf16, tag="small", name=f"ps_ctxT{s}")
        for mc in range(2):
            nc.tensor.transpose(
                psct[:, mc, 0:EAUG],
                ctx_sb[:, P * mc : P * (mc + 1)],
                ident_bf[0:EAUG, 0:EAUG],
            )
        ctxT = small.tile([P, 2, EAUG], bf16, tag="ctxT", name=f"ctxT{s}")
        nc.vector.tensor_copy(ctxT[:], psct[:, :, 0:EAUG])
        ctxT_s[s] = ctxT

    def emit_out_group(s, g):
        # out_un^T tile = ctx_aug^T @ qp^T (ROW 0 = den): 512-col streams
        # against the small ctxT stationary operand.  Normalized in-place in
        # the [e, n] layout (approx reciprocal of the den row at partition 0,
        # broadcast across partitions on GpSimd, one DVE multiply with all
        # operands on partitions 1-64) and DMA'd out as out^T tiles; the
        # final layout permutation happens host-side in the unshard step.
        t, jb = divmod(g, 4)
        psf = ps_fin.tile([EAUG, 512], f32, tag="fin", name=f"ps_fin{s}_{g}")
        for mc in range(2):
            nc.tensor.matmul(
                psf[:],
                lhsT=ctxT_s[s][:, mc, :],
                rhs=qpT_s[s][:, mc, t, 4 * jb : 4 * jb + 4, :],
                start=(mc == 0),
                stop=(mc == 1),
            )
        # psf is double-banked, so the normalize chain reads PSUM directly
        # while the next group's matmuls stream into the other bank
        rec = small.tile([1, 512], f32, tag="rec", name=f"rec{s}_{g}")
        nc.vector.reciprocal_approx_fast(rec[:], psf[0:1, :])
        recb = small.tile([EAUG, 512], f32, tag="recb", name=f"recb{s}_{g}")
        nc.gpsimd.partition_broadcast(recb[:], rec[:], channels=EAUG)
        out_sb = outp.tile([EAUG, 512], f32, tag="ost", name=f"out_sb{s}_{g}")
        nc.vector.tensor_tensor(out_sb[:], psf[:], recb[:], MULT)
        nc.sync.dma_start(out_d[s, g], out_sb[:])

    # ---------------- pipelined stage loop ----------------
    loads = {0: emit_loads(0)}
    for s in range(bhpc + 1):
        cur = s if s < bhpc else None
        prev = s - 1 if s >= 1 else None
        if cur is not None:
            if cur + 1 < bhpc:
                loads[cur + 1] = emit_loads(cur + 1)
            if cur == 0:
                qp0, kp0 = loads.pop(0)
                for w, gg in (("q", 0), ("q", 1), ("k", 0), ("k", 1)):
                    emit_transpose_group(0, qp0 if w == "q" else kp0, w, gg)
            nxt = loads.pop(cur + 1) if cur + 1 < bhpc else None
            emit_qp_block(cur, 0, 0)
            emit_qp_block(cur, 0, 1)
            # next stage's transposes interleave between qp blocks: they only
            # need the DMA'd inputs, and spacing them out keeps the single
            # psum slot chain off the PE queue's critical path
            if nxt is not None:
                emit_transpose_group(cur + 1, nxt[0], "q", 0)
            emit_qp_block(cur, 0, 2)
            emit_qp_block(cur, 0, 3)
            # previous stage's gelu-gated tail, now past its gate
            if prev is not None:
                emit_ctxT(prev)
            emit_qp_block(cur, 1, 0)
            emit_qp_block(cur, 1, 1)
            if nxt is not None:
                emit_transpose_group(cur + 1, nxt[0], "q", 1)
            emit_qp_block(cur, 1, 2)
            emit_qp_block(cur, 1, 3)
            if nxt is not None:
                emit_transpose_group(cur + 1, nxt[1], "k", 0)
            for g in range(8):
                emit_kp_block(cur, g)
                if nxt is not None and g == 0:
                    emit_transpose_group(cur + 1, nxt[1], "k", 1)
                if prev is not None:
                    emit_out_group(prev, g)
        else:
            emit_ctxT(prev)
            for g in range(8):
                emit_out_group(prev, g)

    if repeat > 1:
        loop_cm.__exit__(None, None, None)


def build(bhpc=BHPC, repeat=1):
    from contextlib import ExitStack

    import concourse.mybir as mybir
    import concourse.tile as tile
    from concourse import bacc

    nc = bacc.Bacc("TRN2", target_bir_lowering=False, debug=False)
    f32 = mybir.dt.float32
    q_d = nc.dram_tensor("q", [bhpc, N, D], f32, kind="ExternalInput").ap()
    k_d = nc.dram_tensor("k", [bhpc, N, D], f32, kind="ExternalInput").ap()
    v_d = nc.dram_tensor("v", [bhpc, N, D], f32, kind="ExternalInput").ap()
    proj_d = nc.dram_tensor("proj_mat", [M, D], f32, kind="ExternalInput").ap()
    # out^T tiles [g = t*4+jb, e, (jl, p)]; host permutes to [n, e] in unshard
    out_d = nc.dram_tensor("out", [bhpc, 8, EAUG, 512], f32, kind="ExternalOutput").ap()

    with tile.TileContext(nc) as tc:
        with ExitStack() as body_ctx:
            _emit_body(body_ctx, tc, out_d, q_d, k_d, v_d, proj_d, bhpc, repeat)
    nc.compile()
    return nc


_built = None


def _get_built():
    global _built
    if _built is None:
        _built = build()
    return _built


def _shard_inputs(q, k, v, proj_mat):
    qf = np.ascontiguousarray(q.reshape(BH, N, D), dtype=np.float32)
    kf = np.ascontiguousarray(k.reshape(BH, N, D), dtype=np.float32)
    vf = np.ascontiguousarray(v.reshape(BH, N, D), dtype=np.float32)
    pf = np.ascontiguousarray(proj_mat, dtype=np.float32)
    in_maps = []
    for c in range(NCORES):
        s = slice(c * BHPC, (c + 1) * BHPC)
        in_maps.append({"q": qf[s], "k": kf[s], "v": vf[s], "proj_mat": pf})
    return in_maps


def _unshard_out(raw):
    # raw: [nbh, 8, 65, 512] out^T tiles with g = t*4+jb, cols = (jl, p);
    # row 0 is the (normalized-to-1) den row, rows 1-64 are e = 0..63;
    # n = jb*1024 + jl*256 + t*128 + p
    o = np.asarray(raw).astype(np.float32)[:, :, 1:, :]
    nbh = o.shape[0]
    o = o.reshape(nbh, 2, 4, D, 4, P)
    o = o.transpose(0, 2, 4, 1, 5, 3)  # [bh, jb, jl, t, p, e]
    return np.ascontiguousarray(o.reshape(nbh, N, D))


def run_on_hw(q, k, v, proj_mat, trace=False, **kwargs):
    from concourse.bass_utils import run_bass_kernel_spmd

    nc = _get_built()
    in_maps = _shard_inputs(q, k, v, proj_mat)
    res = run_bass_kernel_spmd(
        nc, in_maps, core_ids=list(range(NCORES)), trace=trace, **kwargs
    )
    out = np.concatenate([r["out"] for r in res.results], axis=0)
    out = _unshard_out(out)
    return out.reshape(B, H, N, D).astype(np.float32), res


def kernel(q, k, v, proj_mat):
    out, _ = run_on_hw(q, k, v, proj_mat, trace=False)
    return out


# revision 24
# speedup vs baseline: 1.0369x; 1.0369x over previous
"""Trainium2 Bass kernel for nn_GeneralizedAttention (Performer-style linear
attention with GELU random features).

Math (per (b,h)):
    qp  = gelu(q @ proj^T)            [n, m]
    kp  = gelu(k @ proj^T)            [n, m]
    ksum= kp.sum(n)                   [m]
    ctx = kp^T @ v                    [m, e]
    den = qp @ ksum                   [n]
    out = (qp @ ctx) / den[:, None]   [n, e]

Sharding: B*H = 64 (b,h) pairs split across 8 cores, 8 pairs each; proj_mat
replicated; no cross-core comms.

The ScalarE gelu stream (2 * n * m elements per pair at 1 elem/cycle/lane,
~18.3 us per pair in 1024-col instructions) is the roofline for this shape.
All engines execute their queues in order, so the emission order is software-
pipelined across (b,h) stages to keep ScalarE saturated:
  - the input transposes for stage s+1 are emitted mid-stage s (they only
    depend on the DMA'd inputs), so the projection matmuls of s+1 are ready
    the moment the last gelu of stage s retires;
  - the final context-accumulation chunk + ctx^T (gated on the last gelu of
    stage s) are deferred past the first qp blocks of stage s+1;
  - the 8 output groups of stage s-1 are interleaved between the kp blocks
    of stage s, filling PE idle windows under the gelu stream.

On-chip layouts per (b,h):
    q^T, k^T as [128, 16, 128] where partition = (t*64 + d), free = (j, p),
    n = j*256 + t*128 + p.  Both 64-row halves are used, so projection matmuls
    issue in (t=0, t=1) pairs on disjoint PE row groups and overlap.
    qp^T is kept [m, n]-major (feeds the final contraction over m),
    kp is kept [n, m]-major (feeds the context contraction over n).
    The ones column PREPENDED to v folds ksum/den into ctx/out as row 0 (the
    approx-reciprocal custom DVE op needs its input at partition 0).
    out^T = ctx_aug^T @ qp^T per 512-col tile (row 0 = den), normalized
    in the [e, n] layout (reciprocal_approx_fast + GpSimd partition
    broadcast + one DVE multiply) and stored as out^T tiles; the final
    [e, n] -> [n, e] layout permutation happens host-side in the unshard.
"""

import numpy as np

B, H, N, D, M = 4, 16, 4096, 64, 256
NCORES = 8
BH = B * H
BHPC = BH // NCORES  # 8 (b,h) pairs per core
P = 128
NJ = N // 256        # 16 pair-blocks of 256 n
NCH = N // P         # 32 chunks of 128 n
EAUG = D + 1         # 65: e plus the folded ksum/den row
EPAD = 66            # 4-byte aligned psum stride


def _emit_body(ctx, tc, out_d, q_d, k_d, v_d, proj_d, bhpc, repeat=1):
    import concourse.bass as bass
    import concourse.mybir as mybir
    from concourse.masks import make_identity

    nc = tc.nc
    f32 = mybir.dt.float32
    bf16 = mybir.dt.bfloat16
    MULT = mybir.AluOpType.mult
    GELU = mybir.ActivationFunctionType.Gelu

    const = ctx.enter_context(tc.tile_pool(name="const", bufs=1))
    inp = ctx.enter_context(tc.tile_pool(name="inp", bufs=6))
    vpool = ctx.enter_context(tc.tile_pool(name="vpool", bufs=3))
    tsb = ctx.enter_context(tc.tile_pool(name="tsb", bufs=6))
    feat = ctx.enter_context(tc.tile_pool(name="feat", bufs=2))
    small = ctx.enter_context(tc.tile_pool(name="small", bufs=3))
    outp = ctx.enter_context(tc.tile_pool(name="outp", bufs=3))
    ps_gen = ctx.enter_context(tc.tile_pool(name="ps_gen", bufs=2, space="PSUM"))
    ps_small = ctx.enter_context(tc.tile_pool(name="ps_small", bufs=1, space="PSUM"))
    ps_acc = ctx.enter_context(tc.tile_pool(name="ps_acc", bufs=1, space="PSUM"))
    ps_fin = ctx.enter_context(tc.tile_pool(name="ps_fin", bufs=2, space="PSUM"))

    ident_bf = const.tile([P, P], bf16, name="ident_bf")
    make_identity(nc, ident_bf)
    ident_f32 = const.tile([P, P], f32, name="ident_f32")
    make_identity(nc, ident_f32)

    # proj^T [d, m] duplicated on both partition halves (rows 0-63 and 64-127)
    proj_nat = const.tile([P, 2, D], f32, name="proj_nat")
    nc.sync.dma_start(proj_nat[:], proj_d.rearrange("(t p) d -> p t d", p=P))
    projT = const.tile([P, M], bf16, name="projT")
    for t in range(2):
        pspt = ps_small.tile([D, P], f32, tag="small", name=f"ps_projT{t}")
        nc.tensor.transpose(pspt[:], proj_nat[:, t, :], ident_f32)
        nc.vector.tensor_copy(projT[0:D, P * t : P * (t + 1)], pspt[:])
        nc.vector.tensor_copy(projT[D:P, P * t : P * (t + 1)], pspt[:])

    if repeat > 1:
        loop_cm = tc.For_i(
            0,
            repeat,
            1,
            hint_engines=(
                mybir.EngineType.PE,
                mybir.EngineType.DVE,
                mybir.EngineType.Activation,
            ),
        )
        loop_cm.__enter__()

    # ---------------- per-stage state ----------------
    qT_s, kT_s = [None] * bhpc, [None] * bhpc
    qpT_s, kp_s = [None] * bhpc, [None] * bhpc
    vaug_s, acc_s = [None] * bhpc, [None] * bhpc
    ctxT_s = [None] * bhpc
    ostage_s = [None] * bhpc

    def emit_loads(s):
        q_pairs = inp.tile([P, NJ, P], bf16, tag="qk", name=f"q_pairs{s}")
        nc.gpsimd.dma_start(
            q_pairs.rearrange("p j (t d) -> p j t d", t=2),
            q_d[s].rearrange("(j t p) d -> p j t d", t=2, p=P),
        )
        k_pairs = inp.tile([P, NJ, P], bf16, tag="qk", name=f"k_pairs{s}")
        nc.gpsimd.dma_start(
            k_pairs.rearrange("p j (t d) -> p j t d", t=2),
            k_d[s].rearrange("(j t p) d -> p j t d", t=2, p=P),
        )
        # ones column FIRST (index 0) so ksum/den land on partition 0 of the
        # ctx/out tiles: reciprocal_approx_fast requires a partition-0 input
        # (custom DVE ops mishandle nonzero input base partitions on HW)
        v_aug = vpool.tile([P, NCH, EAUG], bf16, tag="va", name=f"v_aug{s}")
        nc.gpsimd.memset(v_aug[:, :, 0:1], 1.0)
        nc.gpsimd.dma_start(
            v_aug[:, :, 1:EAUG], v_d[s].rearrange("(c p) d -> p c d", p=P)
        )
        vaug_s[s] = v_aug
        return q_pairs, k_pairs

    def emit_transpose_group(s, pairs, which, g):
        if which == "q" and g == 0:
            qT_s[s] = tsb.tile([P, NJ, P], bf16, tag="t", name=f"qT{s}")
        if which == "k" and g == 0:
            kT_s[s] = tsb.tile([P, NJ, P], bf16, tag="t", name=f"kT{s}")
        dst = qT_s[s] if which == "q" else kT_s[s]
        pst = ps_small.tile([P, 8, P], bf16, tag="small", name=f"ps_t{s}{which}{g}")
        for i in range(8):
            nc.tensor.transpose(pst[:, i, :], pairs[:, 8 * g + i, :], ident_bf)
        nc.vector.tensor_copy(dst[:, 8 * g : 8 * g + 8, :], pst[:])

    def emit_qp_block(s, mc, b4):
        if b4 == 0 and mc == 0:
            qpT_s[s] = feat.tile([P, 2, 2, NJ, P], bf16, tag="qpT", name=f"qpT{s}")
        qpT = qpT_s[s]
        psq = ps_gen.tile([P, 1024], f32, tag="gen", name=f"ps_qp{s}_{mc}{b4}")
        for t in range(2):
            nc.tensor.matmul(
                psq[:, 512 * t : 512 * (t + 1)],
                lhsT=projT[64 * t : 64 * t + 64, P * mc : P * (mc + 1)],
                rhs=qT_s[s][64 * t : 64 * t + 64, 4 * b4 : 4 * b4 + 4, :],
            )
        nc.scalar.activation(qpT[:, mc, :, 4 * b4 : 4 * b4 + 4, :], psq[:], GELU)

    def emit_kp_block(s, g):
        if g == 0:
            kp_s[s] = feat.tile([P, NCH, M], bf16, tag="kp", name=f"kp{s}")
            acc_s[s] = ps_acc.tile([EAUG, M], f32, tag="acc", name=f"ps_ctx{s}")
        kp = kp_s[s]
        kp_v = kp.rearrange("p (j t) m -> p t j m", t=2)
        psk = ps_gen.tile([P, 1024], f32, tag="gen", name=f"ps_kp{s}_{g}")
        for jl in range(2):
            j = 2 * g + jl
            for t in range(2):
                nc.tensor.matmul(
                    psk[:, 256 * (2 * t + jl) : 256 * (2 * t + jl + 1)],
                    lhsT=kT_s[s][64 * t : 64 * t + 64, j, :],
                    rhs=projT[64 * t : 64 * t + 64, :],
                )
        nc.scalar.activation(kp_v[:, :, 2 * g : 2 * g + 2, :], psk[:], GELU)
        # ctx MMs for this block's chunks; the last block's are deferred to
        # the next stage so they don't block the PE queue at the boundary
        if g < 7:
            emit_ctx_chunks(s, 4 * g, 4 * g + 4)

    def emit_ctx_chunks(s, c0, c1):
        for c in range(c0, c1):
            nc.tensor.matmul(
                acc_s[s][:],
                lhsT=vaug_s[s][:, c, :],
                rhs=kp_s[s][:, c, :],
                start=(c == 0),
                stop=(c == NCH - 1),
            )

    def emit_ctxT(s):
        emit_ctx_chunks(s, 28, 32)
        ctx_sb = small.tile([EAUG, M], bf16, tag="ctxsb", name=f"ctx_sb{s}")
        nc.vector.tensor_copy(ctx_sb[:], acc_s[s][:])
        psct = ps_small.tile([P, 2, EPAD], bf16, tag="small", name=f"ps_ctxT{s}")
        for mc in range(2):
            nc.tensor.transpose(
                psct[:, mc, 0:EAUG],
                ctx_sb[:, P * mc : P * (mc + 1)],
                ident_bf[0:EAUG, 0:EAUG],
            )
        ctxT = small.tile([P, 2, EAUG], bf16, tag="ctxT", name=f"ctxT{s}")
        nc.vector.tensor_copy(ctxT[:], psct[:, :, 0:EAUG])
        ctxT_s[s] = ctxT

    def emit_out_group(s, g):
        # out_un^T tile = ctx_aug^T @ qp^T (ROW 0 = den): 512-col streams
        # against the small ctxT stationary operand.  Normalized in-place in
        # the [e, n] layout (approx reciprocal of the den row at partition 0,
        # broadcast across partitions on GpSimd, one DVE multiply with all
        # operands on partitions 1-64) and DMA'd out as out^T tiles; the
        # final layout permutation happens host-side in the unshard step.
        t, jb = divmod(g, 4)
        psf = ps_fin.tile([EAUG, 512], f32, tag="fin", name=f"ps_fin{s}_{g}")
        for mc in range(2):
            nc.tensor.matmul(
                psf[:],
                lhsT=ctxT_s[s][:, mc, :],
                rhs=qpT_s[s][:, mc, t, 4 * jb : 4 * jb + 4, :],
                start=(mc == 0),
                stop=(mc == 1),
            )
        # psf is double-banked, so the normalize chain reads PSUM directly
        # while the next group's matmuls stream into the other bank
        rec = small.tile([1, 512], f32, tag="rec", name=f"rec{s}_{g}")
        nc.vector.reciprocal_approx_fast(rec[:], psf[0:1, :])
        recb = small.tile([EAUG, 512], f32, tag="recb", name=f"recb{s}_{g}")
        nc.gpsimd.partition_broadcast(recb[:], rec[:], channels=EAUG)
        out_sb = outp.tile([EAUG, 512], f32, tag="ost", name=f"out_sb{s}_{g}")
        nc.vector.tensor_tensor(out_sb[:], psf[:], recb[:], MULT)
        nc.sync.dma_start(out_d[s, g], out_sb[:])

    # ---------------- pipelined stage loop ----------------
    loads = {0: emit_loads(0)}
    for s in range(bhpc + 1):
        cur = s if s < bhpc else None
        prev = s - 1 if s >= 1 else None
        if cur is not None:
            if cur + 1 < bhpc:
                loads[cur + 1] = emit_loads(cur + 1)
            if cur == 0:
                qp0, kp0 = loads.pop(0)
                for w, gg in (("q", 0), ("q", 1), ("k", 0), ("k", 1)):
                    emit_transpose_group(0, qp0 if w == "q" else kp0, w, gg)
            nxt = loads.pop(cur + 1) if cur + 1 < bhpc else None
            if cur < bhpc - 1:
                emit_qp_block(cur, 0, 0)
                emit_qp_block(cur, 0, 1)
                # next stage's transposes interleave between qp blocks: they
                # only need the DMA'd inputs, and spacing them out keeps the
                # single psum slot chain off the PE queue's critical path
                if nxt is not None:
                    emit_transpose_group(cur + 1, nxt[0], "q", 0)
                emit_qp_block(cur, 0, 2)
                emit_qp_block(cur, 0, 3)
                # previous stage's gelu-gated tail, now past its gate
                if prev is not None:
                    emit_ctxT(prev)
                emit_qp_block(cur, 1, 0)
                emit_qp_block(cur, 1, 1)
                if nxt is not None:
                    emit_transpose_group(cur + 1, nxt[0], "q", 1)
                emit_qp_block(cur, 1, 2)
                emit_qp_block(cur, 1, 3)
                if nxt is not None:
                    emit_transpose_group(cur + 1, nxt[1], "k", 0)
                for g in range(8):
                    emit_kp_block(cur, g)
                    if nxt is not None and g == 0:
                        emit_transpose_group(cur + 1, nxt[1], "k", 1)
                    if prev is not None:
                        emit_out_group(prev, g)
            else:
                # last stage: kp phase first and qp blocks in (b4, mc) pair
                # order, so the epilogue's out-groups can interleave with qp
                # production instead of draining serially after the final
                # gelu (the loop back-edge is a full barrier, so head+tail
                # drain is paid on every iteration)
                if prev is not None:
                    emit_ctxT(prev)
                for g in range(8):
                    emit_kp_block(cur, g)
                    if prev is not None:
                        emit_out_group(prev, g)
                for b4 in range(4):
                    emit_qp_block(cur, 0, b4)
                    emit_qp_block(cur, 1, b4)
        else:
            emit_ctxT(prev)
            for b4 in range(4):
                for t in range(2):
                    # out group g = t*4 + b4 needs qp pair b4 (both mc)
                    emit_out_group(prev, t * 4 + b4)

    if repeat > 1:
        loop_cm.__exit__(None, None, None)


def build(bhpc=BHPC, repeat=1):
    from contextlib import ExitStack

    import concourse.mybir as mybir
    import concourse.tile as tile
    from concourse import bacc

    nc = bacc.Bacc("TRN2", target_bir_lowering=False, debug=False)
    f32 = mybir.dt.float32
    q_d = nc.dram_tensor("q", [bhpc, N, D], f32, kind="ExternalInput").ap()
    k_d = nc.dram_tensor("k", [bhpc, N, D], f32, kind="ExternalInput").ap()
    v_d = nc.dram_tensor("v", [bhpc, N, D], f32, kind="ExternalInput").ap()
    proj_d = nc.dram_tensor("proj_mat", [M, D], f32, kind="ExternalInput").ap()
    # out^T tiles [g = t*4+jb, e, (jl, p)]; host permutes to [n, e] in unshard
    out_d = nc.dram_tensor("out", [bhpc, 8, EAUG, 512], f32, kind="ExternalOutput").ap()

    with tile.TileContext(nc) as tc:
        with ExitStack() as body_ctx:
            _emit_body(body_ctx, tc, out_d, q_d, k_d, v_d, proj_d, bhpc, repeat)
    nc.compile()
    return nc


_built = None


def _get_built():
    global _built
    if _built is None:
        _built = build()
    return _built


def _shard_inputs(q, k, v, proj_mat):
    qf = np.ascontiguousarray(q.reshape(BH, N, D), dtype=np.float32)
    kf = np.ascontiguousarray(k.reshape(BH, N, D), dtype=np.float32)
    vf = np.ascontiguousarray(v.reshape(BH, N, D), dtype=np.float32)
    pf = np.ascontiguousarray(proj_mat, dtype=np.float32)
    in_maps = []
    for c in range(NCORES):
        s = slice(c * BHPC, (c + 1) * BHPC)
        in_maps.append({"q": qf[s], "k": kf[s], "v": vf[s], "proj_mat": pf})
    return in_maps


def _unshard_out(raw):
    # raw: [nbh, 8, 65, 512] out^T tiles with g = t*4+jb, cols = (jl, p);
    # row 0 is the (normalized-to-1) den row, rows 1-64 are e = 0..63;
    # n = jb*1024 + jl*256 + t*128 + p
    o = np.asarray(raw).astype(np.float32)[:, :, 1:, :]
    nbh = o.shape[0]
    o = o.reshape(nbh, 2, 4, D, 4, P)
    o = o.transpose(0, 2, 4, 1, 5, 3)  # [bh, jb, jl, t, p, e]
    return np.ascontiguousarray(o.reshape(nbh, N, D))


def run_on_hw(q, k, v, proj_mat, trace=False, **kwargs):
    from concourse.bass_utils import run_bass_kernel_spmd

    nc = _get_built()
    in_maps = _shard_inputs(q, k, v, proj_mat)
    res = run_bass_kernel_spmd(
        nc, in_maps, core_ids=list(range(NCORES)), trace=trace, **kwargs
    )
    out = np.concatenate([r["out"] for r in res.results], axis=0)
    out = _unshard_out(out)
    return out.reshape(B, H, N, D).astype(np.float32), res


def kernel(q, k, v, proj_mat):
    out, _ = run_on_hw(q, k, v, proj_mat, trace=False)
    return out
